# revision 1
# baseline (speedup 1.0000x reference)
"""Multi-head causal self-attention (B=2, T=4096, D=768, H=12) on 8 trn2 cores.

Sharding: core c -> batch b = c//4, heads 3*(c%4) .. 3*(c%4)+2.
qkv_proj column-parallel (each core computes Q/K/V only for its heads),
out_proj row-parallel (each core emits a partial y^T; host sums the 4
partials per batch).

Device dataflow (all fp32):
  x^T tiles via PE transposes -> Q^T/K^T via transposed projection
  (W^T stationary, x^T streaming) -> S^T = K Q^T in [k,q] layout, two
  heads row-paired on opposite PE halves -> exp on ScalarE (no max
  subtraction; scores ~ N(0,1)) -> causal band masks on DVE ->
  out^T = V^T P^T col-paired (even k-tiles -> psum partitions 0:64,
  odd -> 64:128) with a parallel 4-way col-tiled ones-matmul computing
  softmax denominators -> normalize via batched reciprocal + gpsimd
  partition broadcast -> y^T = Wo^T.T out^T with heads 0/1 row-paired.
"""

import sys

sys.path.insert(0, "/opt/trn_rl_repo")

import numpy as np
from contextlib import ExitStack

import concourse.bass as bass
import concourse.bacc as bacc
import concourse.tile as tile
import concourse.mybir as mybir
from concourse.masks import make_identity
from concourse.bass_utils import run_bass_kernel_spmd

F32 = mybir.dt.float32
AF = mybir.ActivationFunctionType

B = 2
T = 4096
D = 768
H = 12
DK = 64
NCORES = 8
HL = 3  # heads per core
ND = D // 128  # 6 d-tiles
NKT = T // 128  # 32 k-tiles
NQB = T // 512  # 8 q-blocks
NTSB = T // 512  # 8 t-superblocks (4 t-tiles each)

_CACHE = {}
USE_PB = True  # gpsimd partition_broadcast for the reciprocal broadcast


def _emit(tc):
    nc = tc.nc
    x_d = nc.dram_tensor("x", [T, D], F32, kind="ExternalInput").ap()
    wqk_d = nc.dram_tensor("wqkT", [D, 384], F32, kind="ExternalInput").ap()
    wv_d = nc.dram_tensor("wvT", [D, HL * DK], F32, kind="ExternalInput").ap()
    wo_d = nc.dram_tensor("woT", [HL, DK, D], F32, kind="ExternalInput").ap()
    y_d = nc.dram_tensor("yT", [D, T], F32, kind="ExternalOutput").ap()

    ctx = ExitStack()
    const = ctx.enter_context(tc.tile_pool(name="const", bufs=1))
    persist = ctx.enter_context(tc.tile_pool(name="persist", bufs=1))
    xpool = ctx.enter_context(tc.tile_pool(name="xp", bufs=2))
    xtpool = ctx.enter_context(tc.tile_pool(name="xt", bufs=1))
    ptpool = ctx.enter_context(tc.tile_pool(name="pt", bufs=5))
    spool = ctx.enter_context(tc.tile_pool(name="sp", bufs=2))
    otpool = ctx.enter_context(tc.tile_pool(name="ot", bufs=1))
    ypool = ctx.enter_context(tc.tile_pool(name="yp", bufs=2))
    # PSUM: pa = streaming (S tiles, transposes, qkv, V); pb = AV
    # accumulators (one [128,512] bank per head, even/odd halves);
    # pc = sums accumulators + reduce + yT.
    psA = ctx.enter_context(tc.tile_pool(name="psA", bufs=2, space="PSUM"))
    psB = ctx.enter_context(tc.tile_pool(name="psB", bufs=3, space="PSUM"))
    psC = ctx.enter_context(tc.tile_pool(name="psC", bufs=1, space="PSUM"))

    # ---- constants ----
    ident = const.tile([128, 128], F32)
    make_identity(nc, ident)
    # causal band masks for the 4 diagonal-band k-tiles of each q-block:
    # bandmask[bp][k, q] = 0 for q < 128*bp + k, else 1
    bandmask = []
    for bp in range(4):
        m = const.tile([128, 512], F32, name=f"bandmask{bp}")
        nc.gpsimd.memset(m, 1.0)
        nc.gpsimd.affine_select(
            out=m, in_=m, compare_op=mybir.AluOpType.is_ge, fill=0.0,
            base=-128 * bp, pattern=[[1, 512]], channel_multiplier=-1,
        )
        bandmask.append(m)
    ones1 = const.tile([128, 1], F32)
    nc.vector.memset(ones1, 1.0)
    ones64 = const.tile([1, DK], F32)
    nc.vector.memset(ones64, 1.0)
    ones4 = const.tile([128, 1], F32)
    nc.vector.memset(ones4, 0.0)
    for r in (0, 32, 64, 96):
        nc.vector.memset(ones4[r : r + 1, :], 1.0)

    wqk_sb = const.tile([128, ND, 384], F32)
    nc.sync.dma_start(out=wqk_sb, in_=wqk_d.rearrange("(j p) e -> p j e", p=128))
    wv_sb = const.tile([128, ND, HL * DK], F32)
    nc.sync.dma_start(out=wv_sb, in_=wv_d.rearrange("(j p) e -> p j e", p=128))
    wo01_sb = const.tile([128, D], F32)  # head0 rows on 0:64, head1 on 64:128
    nc.sync.dma_start(out=wo01_sb, in_=wo_d[0:2].rearrange("h p d -> (h p) d"))
    wo2_sb = const.tile([DK, D], F32)
    nc.sync.dma_start(out=wo2_sb, in_=wo_d[2])

    # ---- persistent activations ----
    # KA: [K^T_h0 ; K^T_h1], QB: [Q^T_h0 ; Q^T_h1] on partition halves
    KA = persist.tile([128, T], F32, name="KA")
    QB = persist.tile([128, T], F32, name="QB")
    C2 = persist.tile([128, T], F32, name="C2")  # [K^T_h2 ; Q^T_h2]
    D2 = persist.tile([128, T], F32, name="D2")  # [Q^T_h2 ; K^T_h2] (swapped copy)
    Vh = []
    for h in range(HL):
        vt = persist.tile([128, NKT, DK + 1], F32, name=f"V{h}")
        nc.gpsimd.memset(vt[:, :, DK : DK + 1], 1.0)  # ones row -> softmax sums
        Vh.append(vt)
    ot01 = persist.tile([128, 512], F32, name="ot01")  # heads 0/1 out^T per qb
    ot2 = persist.tile([DK, 512], F32, name="ot2")

    qk_dest = [KA, QB, C2]

    # ================= phase A: projections =================
    for tsb in range(NTSB):
        xt_sb = xtpool.tile([128, ND, 512], F32, name="xt_sb")
        for tt in range(4):
            t0 = (tsb * 4 + tt) * 128
            x_sb = xpool.tile([128, D], F32, name="x_sb")
            nc.sync.dma_start(out=x_sb, in_=x_d[t0 : t0 + 128, :])
            ps_t = psA.tile([128, ND * 128], F32, name="ps_t", tag="pa")
            for dj in range(ND):
                nc.tensor.transpose(
                    ps_t[:, dj * 128 : (dj + 1) * 128],
                    x_sb[:, dj * 128 : (dj + 1) * 128],
                    ident,
                )
            nc.vector.tensor_copy(
                xt_sb[:, :, tt * 128 : (tt + 1) * 128],
                ps_t.rearrange("p (j t) -> p j t", j=ND),
            )
        # Q^T / K^T projection: out[e, t] block per e-tile
        for et in range(3):
            ps_q = psA.tile([128, 512], F32, name="ps_q", tag="pa")
            nc.vector.memset(ps_q, 0.0)
            for dj in range(ND):
                e0 = et * 128
                nc.tensor.matmul(
                    ps_q[0:64, :],
                    lhsT=wqk_sb[:, dj, e0 : e0 + 64],
                    rhs=xt_sb[:, dj, :],
                    start=False, stop=(dj == ND - 1), skip_group_check=True,
                )
                nc.tensor.matmul(
                    ps_q[64:128, :],
                    lhsT=wqk_sb[:, dj, e0 + 64 : e0 + 128],
                    rhs=xt_sb[:, dj, :],
                    start=False, stop=(dj == ND - 1), skip_group_check=True,
                )
            nc.vector.tensor_copy(qk_dest[et][:, tsb * 512 : (tsb + 1) * 512], ps_q)
        # D2 = partition-swapped copy of C2 (for self-paired row-tiling of h2)
        blk = slice(tsb * 512, (tsb + 1) * 512)
        nc.sync.dma_start(out=D2[0:64, blk], in_=C2[64:128, blk])
        nc.sync.dma_start(out=D2[64:128, blk], in_=C2[0:64, blk])
        # V natural: stationary x^T tiles, streaming Wv^T
        for tt in range(4):
            ps_v = psA.tile([128, HL * DK], F32, name="ps_v", tag="pa")
            nc.vector.memset(ps_v, 0.0)
            for dj in range(ND):
                tcol = tt * 128
                nc.tensor.matmul(
                    ps_v[0:64, :],
                    lhsT=xt_sb[:, dj, tcol : tcol + 64],
                    rhs=wv_sb[:, dj, :],
                    start=False, stop=(dj == ND - 1), skip_group_check=True,
                )
                nc.tensor.matmul(
                    ps_v[64:128, :],
                    lhsT=xt_sb[:, dj, tcol + 64 : tcol + 128],
                    rhs=wv_sb[:, dj, :],
                    start=False, stop=(dj == ND - 1), skip_group_check=True,
                )
            kt = tsb * 4 + tt
            for h in range(HL):
                nc.vector.tensor_copy(
                    Vh[h][:, kt, 0:DK], ps_v[:, h * DK : (h + 1) * DK]
                )

    # ================= phase B: attention =================
    for qb in range(NQB):
        nk = 4 * (qb + 1)
        qblk = slice(qb * 512, (qb + 1) * 512)
        ot_slices = {}
        # pass 0: heads (0, 1) row-paired; pass 1: head 2 self-paired
        for hpass, heads in enumerate([(0, 1), (2,)]):
            psav = {h: psB.tile([DK + 1, 512], F32, name=f"psav{h}", tag="pb")
                    for h in heads}
            for kp in range(nk // 2):
                kt0, kt1 = 2 * kp, 2 * kp + 1
                ss = {h: psA.tile([128, 1024], F32, name=f"ss{h}", tag="pa")
                      for h in heads}
                for i, kt in enumerate((kt0, kt1)):
                    kblk = slice(kt * 128, (kt + 1) * 128)
                    off = slice(i * 512, (i + 1) * 512)
                    if hpass == 0:
                        nc.tensor.matmul(
                            ss[0][:, off], lhsT=KA[0:64, kblk],
                            rhs=QB[0:64, qblk], start=True, stop=True,
                        )
                        nc.tensor.matmul(
                            ss[1][:, off], lhsT=KA[64:128, kblk],
                            rhs=QB[64:128, qblk], start=True, stop=True,
                        )
                    elif i == 0:
                        nc.tensor.matmul(
                            ss[2][:, off], lhsT=C2[0:64, kblk],
                            rhs=D2[0:64, qblk], start=True, stop=True,
                        )
                    else:
                        nc.tensor.matmul(
                            ss[2][:, off], lhsT=D2[64:128, kblk],
                            rhs=C2[64:128, qblk], start=True, stop=True,
                        )
                for h in heads:
                    pt = ptpool.tile([128, 1024], F32, name="pt")
                    nc.scalar.activation(pt, ss[h], AF.Exp, scale=0.125)
                    for i, kt in enumerate((kt0, kt1)):
                        off = slice(i * 512, (i + 1) * 512)
                        if kt >= 4 * qb:  # diagonal band tile
                            bp = kt - 4 * qb
                            nc.vector.tensor_mul(
                                pt[:, off], pt[:, off], bandmask[bp]
                            )
                        # AV with the ones row appended to V: row 64 of the
                        # accumulator collects the softmax denominators
                        nc.tensor.matmul(
                            psav[h],
                            lhsT=Vh[h][:, kt, :], rhs=pt[:, off],
                            start=(kt == 0), stop=(kt == nk - 1),
                        )
            # normalize: out^T = (even + odd halves) / sums
            for h in heads:
                sums_sb = spool.tile([1, 512], F32, name="sums_sb")
                nc.vector.tensor_copy(sums_sb, psav[h][DK : DK + 1, :])
                chop = spool.tile([128, 4], F32, name="chop")
                nc.sync.dma_start(out=chop, in_=sums_sb)
                recipC = spool.tile([128, 4], F32, name="recipC")
                nc.vector.reciprocal(recipC, chop)
                recipR = spool.tile([1, 512], F32, name="recipR")
                nc.sync.dma_start(out=recipR, in_=recipC)
                recipb = spool.tile([DK, 512], F32, name="recipb")
                if USE_PB:
                    nc.gpsimd.partition_broadcast(recipb, recipR, channels=DK)
                else:
                    ps_b = psC.tile([128, 512], F32, name="ps_b", tag="pc")
                    nc.tensor.matmul(
                        ps_b[0:DK, :], lhsT=ones64, rhs=recipR,
                        start=True, stop=True,
                    )
                    nc.vector.tensor_copy(recipb, ps_b[0:DK, :])
                if h == 0:
                    nc.vector.tensor_mul(ot01[0:DK, :], psav[h][0:DK, :], recipb)
                    ot_slices[0] = ot01[0:DK, :]
                elif h == 1:
                    ot1s = spool.tile([DK, 512], F32, name="ot1s")
                    nc.vector.tensor_mul(ot1s, psav[h][0:DK, :], recipb)
                    nc.sync.dma_start(out=ot01[DK:128, :], in_=ot1s)
                    ot_slices[1] = ot01[DK:128, :]
                else:
                    nc.vector.tensor_mul(ot2, psav[h][0:DK, :], recipb)
                    ot_slices[2] = ot2
        # out-proj: y^T[d, q] — heads 0/1 stacked on partition halves form a
        # single K=128 contraction; then head 2's K=64 accumulates on top.
        # (Mixed ROW positions inside one accumulation group crash the HW,
        # so never pair row-groups within an accumulating chain.)
        for dj in range(ND):
            dblk = slice(dj * 128, (dj + 1) * 128)
            ps_y = psC.tile([128, 512], F32, name="ps_y", tag="pc")
            nc.tensor.matmul(
                ps_y, lhsT=wo01_sb[:, dblk], rhs=ot01,
                start=True, stop=False, skip_group_check=True,
            )
            nc.tensor.matmul(
                ps_y, lhsT=wo2_sb[:, dblk], rhs=ot2,
                start=False, stop=True, skip_group_check=True,
            )
            y_sb = ypool.tile([128, 512], F32, name="y_sb")
            nc.vector.tensor_copy(y_sb, ps_y)
            nc.sync.dma_start(out=y_d[dblk, qblk], in_=y_sb)
    ctx.close()


def build():
    if "nc" in _CACHE:
        return _CACHE["nc"]
    nc = bacc.Bacc(
        "TRN2", target_bir_lowering=False, debug=False, num_devices=NCORES
    )
    with tile.TileContext(nc) as tc:
        _emit(tc)
    nc.compile()
    _CACHE["nc"] = nc
    return nc


def make_in_maps(x, w_qkv, w_out):
    x = np.asarray(x, dtype=np.float32)
    w_qkv = np.asarray(w_qkv, dtype=np.float32)
    w_out = np.asarray(w_out, dtype=np.float32)
    wq = w_qkv[0:D]        # [768, 768], rows = q features
    wk = w_qkv[D : 2 * D]
    wv = w_qkv[2 * D :]
    in_maps = []
    for c in range(NCORES):
        b, g = divmod(c, 4)
        hs = [3 * g + j for j in range(HL)]  # global head ids
        h0, h1, h2 = hs
        cols = []
        for pair in ((wk, h0), (wk, h1), (wq, h0), (wq, h1), (wk, h2), (wq, h2)):
            w, h = pair
            cols.append(w[h * DK : (h + 1) * DK].T)  # [768, 64]
        wqkT = np.ascontiguousarray(np.concatenate(cols, axis=1))  # [768, 384]
        wvT = np.ascontiguousarray(
            np.concatenate([wv[h * DK : (h + 1) * DK].T for h in hs], axis=1)
        )  # [768, 192]
        woT = np.ascontiguousarray(
            np.stack([w_out[:, h * DK : (h + 1) * DK].T for h in hs])
        )  # [3, 64, 768]
        in_maps.append(
            {
                "x": np.ascontiguousarray(x[b]),
                "wqkT": wqkT,
                "wvT": wvT,
                "woT": woT,
            }
        )
    return in_maps


def run(inputs, trace=False):
    """Run on hardware; returns (y [B,T,D] fp32, BassKernelResults)."""
    nc = build()
    in_maps = make_in_maps(inputs["x"], inputs["w_qkv"], inputs["w_out"])
    br = run_bass_kernel_spmd(nc, in_maps, list(range(NCORES)), trace=trace)
    y = np.zeros((B, T, D), dtype=np.float32)
    for c in range(NCORES):
        b = c // 4
        y[b] += np.asarray(br.results[c]["yT"]).T
    return y, br


def kernel(x, w_qkv, w_out):
    y, _ = run({"x": x, "w_qkv": w_qkv, "w_out": w_out})
    return y



# revision 2
# speedup vs baseline: 2.3499x; 2.3499x over previous
"""Multi-head causal self-attention (B=2, T=4096, D=768, H=12) on 8 trn2 cores.

Sharding: core c -> batch b = c//4, heads 3*(c%4) .. 3*(c%4)+2.
qkv_proj column-parallel (each core computes Q/K/V only for its heads),
out_proj row-parallel (each core emits a partial y^T; host sums the 4
partials per batch).

v2 (bf16): all matmul operands are bf16 (fp32 matmuls run at 1/4 PE rate;
bf16 at full rate), accumulation stays fp32 in PSUM.  x is pre-transposed
on the host so the kernel needs no PE transposes: x^T tiles stream from
DRAM and feed both the Q^T/K^T projection (W^T stationary) and the V
projection (x^T stationary).  Scores S^T = K Q^T are computed per 128-row
k-tile with heads 0/1 row-paired on opposite PE halves (concurrent via
row groups) into a 2-bank PSUM tile; one ScalarE exp per [128,1024] tile
(both heads) keeps the ACT engine saturated - it is the bottleneck at
~1us per k-tile.  Causal band masks multiply on DVE after exp.  AV
accumulates per head with a ones-row appended to V so softmax
denominators fall out as row 64; normalization is DVE reciprocal +
gpsimd partition_broadcast + DVE multiply.  Head 2 runs in a second pass
self-paired via a partition-swapped Q2/K2 copy.  The out-projection of
each q-block is deferred and drip-fed between the next q-block's score
matmuls so the PE never stalls the exp stream.
"""

import sys

sys.path.insert(0, "/opt/trn_rl_repo")

import numpy as np
from contextlib import ExitStack

import concourse.bass as bass
import concourse.bacc as bacc
import concourse.tile as tile
import concourse.mybir as mybir
from concourse.bass_utils import run_bass_kernel_spmd

F32 = mybir.dt.float32
BF16 = mybir.dt.bfloat16
AF = mybir.ActivationFunctionType

B = 2
T = 4096
D = 768
H = 12
DK = 64
NCORES = 8
HL = 3  # heads per core
ND = D // 128  # 6 d-tiles
NKT = T // 128  # 32 k-tiles
NQB = T // 512  # 8 q-blocks

_CACHE = {}


def _emit(tc):
    nc = tc.nc
    xT_d = nc.dram_tensor("xT", [D, T], BF16, kind="ExternalInput").ap()
    wqk_d = nc.dram_tensor("wqkT", [D, 6 * DK], BF16, kind="ExternalInput").ap()
    wv_d = nc.dram_tensor("wvT", [D, HL * DK], BF16, kind="ExternalInput").ap()
    wo01_d = nc.dram_tensor("wo01T", [128, D], BF16, kind="ExternalInput").ap()
    wo2_d = nc.dram_tensor("wo2T", [DK, D], BF16, kind="ExternalInput").ap()
    y_d = nc.dram_tensor("yT", [D, T], F32, kind="ExternalOutput").ap()

    ctx = ExitStack()
    const = ctx.enter_context(tc.tile_pool(name="const", bufs=1))
    persist = ctx.enter_context(tc.tile_pool(name="persist", bufs=1))
    xtpool = ctx.enter_context(tc.tile_pool(name="xt", bufs=2))
    ptpool = ctx.enter_context(tc.tile_pool(name="pt", bufs=4))
    spool = ctx.enter_context(tc.tile_pool(name="sp", bufs=4))
    ypool = ctx.enter_context(tc.tile_pool(name="yp", bufs=2))
    # PSUM: psS = streaming (projections + score tiles), 3x[128,1024] = 6
    # banks; psAV = AV accumulators + out-proj, 2x[128,512] = 2 banks.
    psS = ctx.enter_context(tc.tile_pool(name="psS", bufs=3, space="PSUM"))
    psAV = ctx.enter_context(tc.tile_pool(name="psAV", bufs=2, space="PSUM"))

    # ---- weights ----
    wqk_sb = const.tile([128, ND, 6 * DK], BF16)
    nc.sync.dma_start(out=wqk_sb, in_=wqk_d.rearrange("(j p) e -> p j e", p=128))
    wv_sb = const.tile([128, ND, HL * DK], BF16)
    nc.sync.dma_start(out=wv_sb, in_=wv_d.rearrange("(j p) e -> p j e", p=128))
    wo01_sb = const.tile([128, D], BF16)
    nc.sync.dma_start(out=wo01_sb, in_=wo01_d)
    wo2_sb = const.tile([DK, D], BF16)
    nc.sync.dma_start(out=wo2_sb, in_=wo2_d)

    # warm the exp table set while weights stream in
    warm_in = const.tile([1, 16], F32)
    nc.vector.memset(warm_in, 0.0)
    warm_out = const.tile([1, 16], F32)
    nc.scalar.activation(warm_out, warm_in, AF.Exp, scale=0.125)

    # causal band masks, [k, q]-layout: mask[bp][k, q] = (q >= 128*bp + k).
    # bandA[bp]: same mask in both 512-halves (heads 0/1, same k-tile).
    # bandB[j]:  halves are bp=2j and bp=2j+1 (head 2, k-tile pair).
    def band_fill(m, half, bp):
        nc.gpsimd.affine_select(
            out=m[:, half * 512 : (half + 1) * 512],
            in_=m[:, half * 512 : (half + 1) * 512],
            compare_op=mybir.AluOpType.is_ge, fill=0.0,
            base=-128 * bp, pattern=[[1, 512]], channel_multiplier=-1,
        )

    bandA = []
    for bp in range(4):
        m = const.tile([128, 1024], BF16, name=f"bandA{bp}")
        nc.gpsimd.memset(m, 1.0)
        band_fill(m, 0, bp)
        band_fill(m, 1, bp)
        bandA.append(m)
    bandB = []
    for j in range(2):
        m = const.tile([128, 1024], BF16, name=f"bandB{j}")
        nc.gpsimd.memset(m, 1.0)
        band_fill(m, 0, 2 * j)
        band_fill(m, 1, 2 * j + 1)
        bandB.append(m)

    # ---- persistent activations ----
    # KA: [K^T_h0 ; K^T_h1], QB: [Q^T_h0 ; Q^T_h1] on partition halves
    KA = persist.tile([128, T], BF16, name="KA")
    QB = persist.tile([128, T], BF16, name="QB")
    C2 = persist.tile([128, T], BF16, name="C2")  # [K^T_h2 ; Q^T_h2]
    D2 = persist.tile([128, T], BF16, name="D2")  # [Q^T_h2 ; K^T_h2] (swapped)
    # V natural [t, e] per k-tile with a ones col at e=64 -> softmax sums
    Vall = persist.tile([128, NKT, HL, DK + 1], BF16, name="Vall")
    nc.gpsimd.memset(Vall[:, :, :, DK : DK + 1], 1.0)
    ot01 = persist.tile([128, 512], BF16, name="ot01")  # heads 0/1 out^T
    ot2 = persist.tile([DK, 512], BF16, name="ot2")

    qk_dest = [KA, QB, C2]

    # ================= phase A: projections =================
    for tsb in range(NQB):
        tblk = slice(tsb * 512, (tsb + 1) * 512)
        xt_sb = xtpool.tile([128, ND, 512], BF16, name="xt_sb")
        nc.sync.dma_start(
            out=xt_sb, in_=xT_d[:, tblk].rearrange("(j p) t -> p j t", p=128)
        )
        # Q^T / K^T projection: out[e, t] block per e-tile
        for et in range(3):
            ps_q = psS.tile([128, 512], F32, name="ps_q", tag="ps")
            for dj in range(ND):
                nc.tensor.matmul(
                    ps_q,
                    lhsT=wqk_sb[:, dj, et * 128 : (et + 1) * 128],
                    rhs=xt_sb[:, dj, :],
                    start=(dj == 0), stop=(dj == ND - 1),
                )
            nc.vector.tensor_copy(qk_dest[et][:, tblk], ps_q)
        # D2 = partition-swapped copy of C2 (for self-paired tiling of h2)
        nc.sync.dma_start(out=D2[0:64, tblk], in_=C2[64:128, tblk])
        nc.sync.dma_start(out=D2[64:128, tblk], in_=C2[0:64, tblk])
        # V natural: stationary x^T tiles, streaming Wv^T
        for tt in range(4):
            kt = tsb * 4 + tt
            ps_v = psS.tile([128, HL * DK], F32, name="ps_v", tag="ps")
            for dj in range(ND):
                nc.tensor.matmul(
                    ps_v,
                    lhsT=xt_sb[:, dj, tt * 128 : (tt + 1) * 128],
                    rhs=wv_sb[:, dj, :],
                    start=(dj == 0), stop=(dj == ND - 1),
                )
            nc.vector.tensor_copy(
                Vall[:, kt, :, 0:DK], ps_v.rearrange("p (h e) -> p h e", h=HL)
            )

    # ================= phase B: attention =================
    def normalize(av, dest):
        """dest[e, q] = av[e, q] / av[64, q] (softmax denominators)."""
        recip = spool.tile([1, 512], F32, name="recip")
        nc.vector.reciprocal(recip, av[DK : DK + 1, :])
        recipb = spool.tile([DK, 512], F32, name="recipb")
        nc.gpsimd.partition_broadcast(recipb, recip, channels=DK)
        nc.vector.tensor_mul(dest, av[0:DK, :], recipb)

    pending_out = []  # deferred out-proj thunks from the previous q-block

    for qb in range(NQB):
        nk = 4 * (qb + 1)
        qblk = slice(qb * 512, (qb + 1) * 512)

        # ---- pass 0: heads 0/1, row-paired on opposite PE halves ----
        av0 = psAV.tile([DK + 1, 512], F32, name="av0", tag="av")
        av1 = psAV.tile([DK + 1, 512], F32, name="av1", tag="av")
        avs = [av0, av1]

        def scores01(kt):
            pss = psS.tile([128, 1024], F32, name="pss", tag="ps")
            kblk = slice(kt * 128, (kt + 1) * 128)
            nc.tensor.matmul(
                pss[:, 0:512], lhsT=KA[0:64, kblk], rhs=QB[0:64, qblk],
                start=True, stop=True,
            )
            nc.tensor.matmul(
                pss[:, 512:1024], lhsT=KA[64:128, kblk], rhs=QB[64:128, qblk],
                start=True, stop=True,
            )
            return pss

        pss_next = scores01(0)
        for kt in range(nk):
            pss_cur = pss_next
            if kt + 1 < nk:
                pss_next = scores01(kt + 1)
            # drip-feed the previous q-block's out-projection between the
            # score matmuls so it never starves the exp stream
            if pending_out:
                pending_out.pop(0)()
            pt = ptpool.tile([128, 1024], BF16, name="pt")
            nc.scalar.activation(pt, pss_cur, AF.Exp, scale=0.125)
            if kt >= 4 * qb:
                nc.vector.tensor_mul(pt, pt, bandA[kt - 4 * qb])
            for h in (0, 1):
                nc.tensor.matmul(
                    avs[h],
                    lhsT=Vall[:, kt, h, :], rhs=pt[:, h * 512 : (h + 1) * 512],
                    start=(kt == 0), stop=(kt == nk - 1),
                )
        normalize(av0, ot01[0:DK, :])
        ot1s = spool.tile([DK, 512], BF16, name="ot1s")
        normalize(av1, ot1s)
        nc.sync.dma_start(out=ot01[DK:128, :], in_=ot1s)

        # ---- pass 1: head 2, self-paired k-tile pairs via C2/D2 ----
        av2 = psAV.tile([DK + 1, 512], F32, name="av2", tag="av")

        def scores2(kp):
            pss = psS.tile([128, 1024], F32, name="pss2", tag="ps")
            b0 = slice((2 * kp) * 128, (2 * kp + 1) * 128)
            b1 = slice((2 * kp + 1) * 128, (2 * kp + 2) * 128)
            nc.tensor.matmul(
                pss[:, 0:512], lhsT=C2[0:64, b0], rhs=D2[0:64, qblk],
                start=True, stop=True,
            )
            nc.tensor.matmul(
                pss[:, 512:1024], lhsT=D2[64:128, b1], rhs=C2[64:128, qblk],
                start=True, stop=True,
            )
            return pss

        nkp = nk // 2
        pss_next = scores2(0)
        for kp in range(nkp):
            pss_cur = pss_next
            if kp + 1 < nkp:
                pss_next = scores2(kp + 1)
            if pending_out:
                pending_out.pop(0)()
            pt2 = ptpool.tile([128, 1024], BF16, name="pt2")
            nc.scalar.activation(pt2, pss_cur, AF.Exp, scale=0.125)
            if 2 * kp >= 4 * qb:
                nc.vector.tensor_mul(pt2, pt2, bandB[kp - 2 * qb])
            nc.tensor.matmul(
                av2, lhsT=Vall[:, 2 * kp, 2, :], rhs=pt2[:, 0:512],
                start=(kp == 0), stop=False,
            )
            nc.tensor.matmul(
                av2, lhsT=Vall[:, 2 * kp + 1, 2, :], rhs=pt2[:, 512:1024],
                start=False, stop=(kp == nkp - 1),
            )
        normalize(av2, ot2)

        # ---- out-proj: y^T[d, q] = Wo01^T.T ot01 + Wo2^T.T ot2 ----
        def make_outproj(dj, qblk=qblk):
            def thunk():
                dblk = slice(dj * 128, (dj + 1) * 128)
                psy = psAV.tile([128, 512], F32, name="psy", tag="av")
                nc.tensor.matmul(
                    psy, lhsT=wo01_sb[:, dblk], rhs=ot01,
                    start=True, stop=False, skip_group_check=True,
                )
                nc.tensor.matmul(
                    psy, lhsT=wo2_sb[:, dblk], rhs=ot2,
                    start=False, stop=True, skip_group_check=True,
                )
                y_sb = ypool.tile([128, 512], F32, name="y_sb")
                nc.vector.tensor_copy(y_sb, psy)
                nc.sync.dma_start(out=y_d[dblk, qblk], in_=y_sb)
            return thunk

        pending_out.extend(make_outproj(dj) for dj in range(ND))

    for thunk in pending_out:
        thunk()
    ctx.close()


def build():
    if "nc" in _CACHE:
        return _CACHE["nc"]
    nc = bacc.Bacc(
        "TRN2", target_bir_lowering=False, debug=False, num_devices=NCORES
    )
    with tile.TileContext(nc) as tc:
        _emit(tc)
    nc.compile()
    _CACHE["nc"] = nc
    return nc


def make_in_maps(x, w_qkv, w_out):
    import ml_dtypes

    bf16 = ml_dtypes.bfloat16
    x = np.asarray(x, dtype=np.float32)
    w_qkv = np.asarray(w_qkv, dtype=np.float32)
    w_out = np.asarray(w_out, dtype=np.float32)
    wq = w_qkv[0:D]        # [768, 768], rows = q features
    wk = w_qkv[D : 2 * D]
    wv = w_qkv[2 * D :]
    in_maps = []
    for c in range(NCORES):
        b, g = divmod(c, 4)
        hs = [3 * g + j for j in range(HL)]  # global head ids
        h0, h1, h2 = hs
        cols = []
        for pair in ((wk, h0), (wk, h1), (wq, h0), (wq, h1), (wk, h2), (wq, h2)):
            w, h = pair
            cols.append(w[h * DK : (h + 1) * DK].T)  # [768, 64]
        wqkT = np.concatenate(cols, axis=1).astype(bf16)  # [768, 384]
        wvT = np.concatenate(
            [wv[h * DK : (h + 1) * DK].T for h in hs], axis=1
        ).astype(bf16)  # [768, 192]
        wo01T = np.concatenate(
            [w_out[:, h * DK : (h + 1) * DK].T for h in (h0, h1)], axis=0
        ).astype(bf16)  # [128, 768]
        wo2T = w_out[:, h2 * DK : (h2 + 1) * DK].T.astype(bf16)  # [64, 768]
        xT = np.ascontiguousarray(x[b].T).astype(bf16)  # [768, 4096]
        in_maps.append(
            {"xT": xT, "wqkT": wqkT, "wvT": wvT, "wo01T": wo01T, "wo2T": wo2T}
        )
    return in_maps


def run(inputs, trace=False):
    """Run on hardware; returns (y [B,T,D] fp32, BassKernelResults)."""
    nc = build()
    in_maps = make_in_maps(inputs["x"], inputs["w_qkv"], inputs["w_out"])
    br = run_bass_kernel_spmd(nc, in_maps, list(range(NCORES)), trace=trace)
    y = np.zeros((B, T, D), dtype=np.float32)
    for c in range(NCORES):
        b = c // 4
        y[b] += np.asarray(br.results[c]["yT"]).T
    return y, br


def kernel(x, w_qkv, w_out):
    y, _ = run({"x": x, "w_qkv": w_qkv, "w_out": w_out})
    return y


# revision 4
# speedup vs baseline: 3.1121x; 1.3243x over previous
"""Multi-head causal self-attention (B=2, T=4096, D=768, H=12) on 8 trn2 cores.

Sharding: core c -> batch b = c//4, heads 3*(c%4) .. 3*(c%4)+2.
qkv_proj column-parallel (each core computes Q/K/V only for its heads),
out_proj row-parallel (each core emits a partial y^T; host sums the 4
partials per batch).

v3 (bf16, ACT-saturated): all matmul operands are bf16 (fp32 matmuls run
at 1/4 PE rate; bf16 at full rate), accumulation stays fp32 in PSUM.
x is pre-transposed on the host so the kernel needs no PE transposes.
The ScalarE exp stream is the bottleneck (~1us per [128,1024] score
tile, ~215us total), so everything else is scheduled to hide inside it:

- scores S^T = K Q^T per 128-row k-tile, heads 0/1 row-paired on
  opposite PE halves (concurrent via row groups) into a 2-bank PSUM
  tile; one exp covers both heads.  Head 2 runs in a second pass
  self-paired via a partition-swapped Q2/K2 copy.
- score matmuls are software-pipelined one k-tile ahead of the exp.
- AV accumulates per head with a ones-row appended to V so softmax
  denominators fall out as row 64.  Dedicated PSUM slots per head
  (tag "av", bufs=3) so head-2 accumulation never waits on the
  head-0/1 normalize.
- normalization: denominators are DMA-reshaped [1,512]->[128,4] so the
  DVE reciprocal runs across 128 lanes (a [1,512] reciprocal costs
  3.3us on one lane), then gpsimd partition_broadcast + DVE multiply.
- phase A (projections) for t-block 0 runs upfront; projections for
  block qb+1 and the out-projection of block qb-1 are drip-fed one
  chunk per k-tile into block qb's attention loop so the PE never
  starves the exp stream.
"""

import sys

sys.path.insert(0, "/opt/trn_rl_repo")

import numpy as np
from contextlib import ExitStack

import concourse.bass as bass
import concourse.bacc as bacc
import concourse.tile as tile
import concourse.mybir as mybir
from concourse.bass_utils import run_bass_kernel_spmd

F32 = mybir.dt.float32
BF16 = mybir.dt.bfloat16
AF = mybir.ActivationFunctionType

B = 2
T = 4096
D = 768
H = 12
DK = 64
NCORES = 8
HL = 3  # heads per core
ND = D // 128  # 6 d-tiles
NKT = T // 128  # 32 k-tiles
NQB = T // 512  # 8 q-blocks

_CACHE = {}


def _emit(tc):
    nc = tc.nc
    xT_d = nc.dram_tensor("xT", [D, T], BF16, kind="ExternalInput").ap()
    wqk_d = nc.dram_tensor("wqkT", [D, 6 * DK], BF16, kind="ExternalInput").ap()
    wv_d = nc.dram_tensor("wvT", [D, HL * DK], BF16, kind="ExternalInput").ap()
    wo01_d = nc.dram_tensor("wo01T", [128, D], BF16, kind="ExternalInput").ap()
    wo2_d = nc.dram_tensor("wo2T", [DK, D], BF16, kind="ExternalInput").ap()
    y_d = nc.dram_tensor("yT", [D, T], F32, kind="ExternalOutput").ap()

    ctx = ExitStack()
    const = ctx.enter_context(tc.tile_pool(name="const", bufs=1))
    persist = ctx.enter_context(tc.tile_pool(name="persist", bufs=1))
    xtpool = ctx.enter_context(tc.tile_pool(name="xt", bufs=2))
    ptpool = ctx.enter_context(tc.tile_pool(name="pt", bufs=6))
    spool = ctx.enter_context(tc.tile_pool(name="sp", bufs=6))
    ypool = ctx.enter_context(tc.tile_pool(name="yp", bufs=2))
    # PSUM (8 banks): psS tag "ps" 2x[128,1024] = 4 banks (score tiles,
    # double-buffered for the exp stream); psAV tag "av" 3x[65,512] = 3
    # banks (one AV accumulator per head); tag "x" 1x[128,512] = 1 bank
    # (projection chunks + out-proj, strictly serialized drip-feed work).
    psS = ctx.enter_context(tc.tile_pool(name="psS", bufs=2, space="PSUM"))
    psAV = ctx.enter_context(tc.tile_pool(name="psAV", bufs=1, space="PSUM"))

    # ---- weights ----
    wqk_sb = const.tile([128, ND, 6 * DK], BF16)
    nc.sync.dma_start(out=wqk_sb, in_=wqk_d.rearrange("(j p) e -> p j e", p=128))
    wv_sb = const.tile([128, ND, HL * DK], BF16)
    nc.sync.dma_start(out=wv_sb, in_=wv_d.rearrange("(j p) e -> p j e", p=128))
    wo01_sb = const.tile([128, D], BF16)
    nc.sync.dma_start(out=wo01_sb, in_=wo01_d)
    wo2_sb = const.tile([DK, D], BF16)
    nc.sync.dma_start(out=wo2_sb, in_=wo2_d)

    # warm the exp table set while weights stream in
    warm_in = const.tile([1, 16], F32)
    nc.vector.memset(warm_in, 0.0)
    warm_out = const.tile([1, 16], F32)
    nc.scalar.activation(warm_out, warm_in, AF.Exp, scale=0.125)

    # causal band masks, [k, q]-layout: mask[bp][k, q] = (q >= 128*bp + k).
    # bandA[bp]: same mask in both 512-halves (heads 0/1, same k-tile).
    # bandB[j]:  halves are bp=2j and bp=2j+1 (head 2, k-tile pair).
    def band_fill(m, half, bp):
        nc.gpsimd.affine_select(
            out=m[:, half * 512 : (half + 1) * 512],
            in_=m[:, half * 512 : (half + 1) * 512],
            compare_op=mybir.AluOpType.is_ge, fill=0.0,
            base=-128 * bp, pattern=[[1, 512]], channel_multiplier=-1,
        )

    bandA = []
    for bp in range(4):
        m = const.tile([128, 1024], BF16, name=f"bandA{bp}")
        nc.gpsimd.memset(m, 1.0)
        band_fill(m, 0, bp)
        band_fill(m, 1, bp)
        bandA.append(m)
    bandB = []
    for j in range(2):
        m = const.tile([128, 1024], BF16, name=f"bandB{j}")
        nc.gpsimd.memset(m, 1.0)
        band_fill(m, 0, 2 * j)
        band_fill(m, 1, 2 * j + 1)
        bandB.append(m)

    # ---- persistent activations ----
    # KA: [K^T_h0 ; K^T_h1], QB: [Q^T_h0 ; Q^T_h1] on partition halves
    KA = persist.tile([128, T], BF16, name="KA")
    QB = persist.tile([128, T], BF16, name="QB")
    C2 = persist.tile([128, T], BF16, name="C2")  # [K^T_h2 ; Q^T_h2]
    D2 = persist.tile([128, T], BF16, name="D2")  # [Q^T_h2 ; K^T_h2] (swapped)
    # V natural [t, e] per k-tile with a ones col at e=64 -> softmax sums
    Vall = persist.tile([128, NKT, HL, DK + 1], BF16, name="Vall")
    nc.gpsimd.memset(Vall[:, :, :, DK : DK + 1], 1.0)
    ot01 = persist.tile([128, 512], BF16, name="ot01")  # heads 0/1 out^T
    ot2 = persist.tile([DK, 512], BF16, name="ot2")

    qk_dest = [KA, QB, C2]
    xts = {}

    def emit_xt_dma(tsb):
        tblk = slice(tsb * 512, (tsb + 1) * 512)
        xt_sb = xtpool.tile([128, ND, 512], BF16, name="xt_sb")
        nc.sync.dma_start(
            out=xt_sb, in_=xT_d[:, tblk].rearrange("(j p) t -> p j t", p=128)
        )
        xts[tsb] = xt_sb

    def proj_chunks(tsb):
        """PE chunk thunks for t-superblock tsb's projections (~0.6-1.3us
        of PE work each), drip-fed between attention k-tiles."""
        tblk = slice(tsb * 512, (tsb + 1) * 512)

        def c_q(et):
            def thunk():
                xt_sb = xts[tsb]
                ps_q = psAV.tile([128, 512], F32, name="ps_q", tag="x")
                for dj in range(ND):
                    nc.tensor.matmul(
                        ps_q,
                        lhsT=wqk_sb[:, dj, et * 128 : (et + 1) * 128],
                        rhs=xt_sb[:, dj, :],
                        start=(dj == 0), stop=(dj == ND - 1),
                    )
                nc.vector.tensor_copy(qk_dest[et][:, tblk], ps_q)
                if et == 2:
                    # D2 = partition-swapped C2 (self-paired tiling of h2)
                    nc.sync.dma_start(out=D2[0:64, tblk], in_=C2[64:128, tblk])
                    nc.sync.dma_start(out=D2[64:128, tblk], in_=C2[0:64, tblk])
            return thunk

        def c_v(tt):
            def thunk():
                xt_sb = xts[tsb]
                kt = tsb * 4 + tt
                ps_v = psAV.tile([128, HL * DK], F32, name="ps_v", tag="x")
                for dj in range(ND):
                    nc.tensor.matmul(
                        ps_v,
                        lhsT=xt_sb[:, dj, tt * 128 : (tt + 1) * 128],
                        rhs=wv_sb[:, dj, :],
                        start=(dj == 0), stop=(dj == ND - 1),
                    )
                nc.vector.tensor_copy(
                    Vall[:, kt, :, 0:DK],
                    ps_v.rearrange("p (h e) -> p h e", h=HL),
                )
            return thunk

        return [c_q(0), c_q(1), c_q(2), c_v(0), c_v(1), c_v(2), c_v(3)]

    # phase A for t-block 0 runs upfront (nothing to hide it behind)
    emit_xt_dma(0)
    if NQB > 1:
        emit_xt_dma(1)
    for thunk in proj_chunks(0):
        thunk()

    # ================= attention =================
    def normalize(av, dest):
        """dest[e, q] = av[e, q] / av[64, q] (softmax denominators).
        Reshape the sums through DMA so the reciprocal runs across 128
        lanes instead of one."""
        sums_sb = spool.tile([1, 512], F32, name="sums_sb")
        nc.vector.tensor_copy(sums_sb, av[DK : DK + 1, :])
        chop = spool.tile([128, 4], F32, name="chop")
        nc.sync.dma_start(out=chop, in_=sums_sb)
        recipC = spool.tile([128, 4], F32, name="recipC")
        nc.vector.reciprocal(recipC, chop)
        recipR = spool.tile([1, 512], F32, name="recipR")
        nc.sync.dma_start(out=recipR, in_=recipC)
        recipb = spool.tile([DK, 512], F32, name="recipb")
        nc.gpsimd.partition_broadcast(recipb, recipR, channels=DK)
        nc.vector.tensor_mul(dest, av[0:DK, :], recipb)

    pending = []  # drip-feed thunks: out-proj of qb-1, projections of qb+1

    for qb in range(NQB):
        nk = 4 * (qb + 1)
        qblk = slice(qb * 512, (qb + 1) * 512)
        if qb + 2 < NQB:
            emit_xt_dma(qb + 2)  # prefetch x^T for the block after next
        if qb + 1 < NQB:
            pending.extend(proj_chunks(qb + 1))

        # ---- pass 0: heads 0/1, row-paired on opposite PE halves ----
        av0 = psAV.tile([DK + 1, 512], F32, name="av0", tag="av", bufs=3)
        av1 = psAV.tile([DK + 1, 512], F32, name="av1", tag="av", bufs=3)
        avs = [av0, av1]

        def scores01(kt):
            pss = psS.tile([128, 1024], F32, name="pss", tag="ps")
            kblk = slice(kt * 128, (kt + 1) * 128)
            nc.tensor.matmul(
                pss[:, 0:512], lhsT=KA[0:64, kblk], rhs=QB[0:64, qblk],
                start=True, stop=True,
            )
            nc.tensor.matmul(
                pss[:, 512:1024], lhsT=KA[64:128, kblk], rhs=QB[64:128, qblk],
                start=True, stop=True,
            )
            return pss

        pss_next = scores01(0)
        for kt in range(nk):
            pss_cur = pss_next
            if kt + 1 < nk:
                pss_next = scores01(kt + 1)
            if pending:
                pending.pop(0)()
            pt = ptpool.tile([128, 1024], BF16, name="pt")
            nc.scalar.activation(pt, pss_cur, AF.Exp, scale=0.125)
            if kt >= 4 * qb:
                nc.vector.tensor_mul(pt, pt, bandA[kt - 4 * qb])
            for h in (0, 1):
                nc.tensor.matmul(
                    avs[h],
                    lhsT=Vall[:, kt, h, :], rhs=pt[:, h * 512 : (h + 1) * 512],
                    start=(kt == 0), stop=(kt == nk - 1),
                )

        # ---- pass 1: head 2, self-paired k-tile pairs via C2/D2 ----
        av2 = psAV.tile([DK + 1, 512], F32, name="av2", tag="av", bufs=3)

        def scores2(kp):
            pss = psS.tile([128, 1024], F32, name="pss2", tag="ps")
            b0 = slice((2 * kp) * 128, (2 * kp + 1) * 128)
            b1 = slice((2 * kp + 1) * 128, (2 * kp + 2) * 128)
            nc.tensor.matmul(
                pss[:, 0:512], lhsT=C2[0:64, b0], rhs=D2[0:64, qblk],
                start=True, stop=True,
            )
            nc.tensor.matmul(
                pss[:, 512:1024], lhsT=D2[64:128, b1], rhs=C2[64:128, qblk],
                start=True, stop=True,
            )
            return pss

        nkp = nk // 2
        pss_next = scores2(0)
        for kp in range(nkp):
            pss_cur = pss_next
            if kp + 1 < nkp:
                pss_next = scores2(kp + 1)
            if pending:
                pending.pop(0)()
            pt2 = ptpool.tile([128, 1024], BF16, name="pt2")
            nc.scalar.activation(pt2, pss_cur, AF.Exp, scale=0.125)
            if 2 * kp >= 4 * qb:
                nc.vector.tensor_mul(pt2, pt2, bandB[kp - 2 * qb])
            nc.tensor.matmul(
                av2, lhsT=Vall[:, 2 * kp, 2, :], rhs=pt2[:, 0:512],
                start=(kp == 0), stop=False,
            )
            nc.tensor.matmul(
                av2, lhsT=Vall[:, 2 * kp + 1, 2, :], rhs=pt2[:, 512:1024],
                start=False, stop=(kp == nkp - 1),
            )

        # flush leftovers so out-proj of qb-1 is done before ot01 rewrite
        while pending:
            pending.pop(0)()

        normalize(av0, ot01[0:DK, :])
        ot1s = spool.tile([DK, 512], BF16, name="ot1s")
        normalize(av1, ot1s)
        nc.sync.dma_start(out=ot01[DK:128, :], in_=ot1s)
        normalize(av2, ot2)

        # ---- out-proj: y^T[d, q] = Wo01^T.T ot01 + Wo2^T.T ot2 ----
        def make_outproj(dj, qblk=qblk):
            def thunk():
                dblk = slice(dj * 128, (dj + 1) * 128)
                psy = psAV.tile([128, 512], F32, name="psy", tag="x")
                nc.tensor.matmul(
                    psy, lhsT=wo01_sb[:, dblk], rhs=ot01,
                    start=True, stop=False, skip_group_check=True,
                )
                nc.tensor.matmul(
                    psy, lhsT=wo2_sb[:, dblk], rhs=ot2,
                    start=False, stop=True, skip_group_check=True,
                )
                y_sb = ypool.tile([128, 512], F32, name="y_sb")
                nc.vector.tensor_copy(y_sb, psy)
                nc.sync.dma_start(out=y_d[dblk, qblk], in_=y_sb)
            return thunk

        pending.extend(make_outproj(dj) for dj in range(ND))

    for thunk in pending:
        thunk()
    ctx.close()


def build():
    if "nc" in _CACHE:
        return _CACHE["nc"]
    nc = bacc.Bacc(
        "TRN2", target_bir_lowering=False, debug=False, num_devices=NCORES
    )
    with tile.TileContext(nc) as tc:
        _emit(tc)
    nc.compile()
    _CACHE["nc"] = nc
    return nc


def make_in_maps(x, w_qkv, w_out):
    import ml_dtypes

    bf16 = ml_dtypes.bfloat16
    x = np.asarray(x, dtype=np.float32)
    w_qkv = np.asarray(w_qkv, dtype=np.float32)
    w_out = np.asarray(w_out, dtype=np.float32)
    wq = w_qkv[0:D]        # [768, 768], rows = q features
    wk = w_qkv[D : 2 * D]
    wv = w_qkv[2 * D :]
    in_maps = []
    for c in range(NCORES):
        b, g = divmod(c, 4)
        hs = [3 * g + j for j in range(HL)]  # global head ids
        h0, h1, h2 = hs
        cols = []
        for pair in ((wk, h0), (wk, h1), (wq, h0), (wq, h1), (wk, h2), (wq, h2)):
            w, h = pair
            cols.append(w[h * DK : (h + 1) * DK].T)  # [768, 64]
        wqkT = np.concatenate(cols, axis=1).astype(bf16)  # [768, 384]
        wvT = np.concatenate(
            [wv[h * DK : (h + 1) * DK].T for h in hs], axis=1
        ).astype(bf16)  # [768, 192]
        wo01T = np.concatenate(
            [w_out[:, h * DK : (h + 1) * DK].T for h in (h0, h1)], axis=0
        ).astype(bf16)  # [128, 768]
        wo2T = w_out[:, h2 * DK : (h2 + 1) * DK].T.astype(bf16)  # [64, 768]
        xT = np.ascontiguousarray(x[b].T).astype(bf16)  # [768, 4096]
        in_maps.append(
            {"xT": xT, "wqkT": wqkT, "wvT": wvT, "wo01T": wo01T, "wo2T": wo2T}
        )
    return in_maps


def run(inputs, trace=False):
    """Run on hardware; returns (y [B,T,D] fp32, BassKernelResults)."""
    nc = build()
    in_maps = make_in_maps(inputs["x"], inputs["w_qkv"], inputs["w_out"])
    br = run_bass_kernel_spmd(nc, in_maps, list(range(NCORES)), trace=trace)
    y = np.zeros((B, T, D), dtype=np.float32)
    for c in range(NCORES):
        b = c // 4
        y[b] += np.asarray(br.results[c]["yT"]).T
    return y, br


def kernel(x, w_qkv, w_out):
    y, _ = run({"x": x, "w_qkv": w_qkv, "w_out": w_out})
    return y
